# revision 1
# baseline (speedup 1.0000x reference)
"""Trainium2 Bass kernel for nn_AngularDescriptor (gnn_message_passing).

Algorithm: the reference's O(N*M^2) triplet sum is factorized with the
Legendre addition theorem  P_l(u.v) = sum_m Yhat_lm(u) Yhat_lm(v):

  q[i,d,l] = 0.5 * ( sum_{m in shell l} A[i,d,m]^2  -  B[i,d] )
  A[i,d,m] = sum_j g_ij[d] * Yhat_m(u_ij),   B[i,d] = sum_j g_ij[d]^2

so only N*M pairs are ever materialized.  Sharding: atoms split across the
8 NeuronCores (1280/core, layout [128 partitions x 10 atom slots]); per-pair
neighbor records and c_table rows are fetched with GPSIMD dma_gather using
host-marshaled int16 index tiles (16-partition wrapped layout).
"""
import os
import sys

sys.path.insert(0, "/opt/trn_rl_repo")
os.environ.setdefault("NEURON_RT_RESET_CORES", "1")

import math
import numpy as np

from concourse import bacc, bass, mybir, tile
from concourse.bass_utils import run_bass_kernel_spmd

# problem constants (hardcoded per harness rules)
N_ATOMS = 10000
M_NBR = 20
N_TYPES = 4
N_DESC = 8
K_MAX = 8
L_MAX = 4
R_C = 5.0

NCORES = 8
P = 128
S = 10                       # atom slots per partition
CA = P * S                   # atoms per core = 1280
NTOT = NCORES * CA           # padded atom count = 10240
PAIRS = S * M_NBR            # 200 pair slots per partition
NIDX = P * PAIRS             # 25600 gathered records per core
REC = 64                     # f32 elems per gather record (256B)
F32 = mybir.dt.float32
I16 = mybir.dt.int16

SQ3 = math.sqrt(3.0)
C31 = math.sqrt(3.0 / 8.0)
C32 = math.sqrt(15.0)
C33 = math.sqrt(5.0 / 8.0)
SHELL_OFF = [0, 1, 4, 9, 16]


def _ap(t, off, dims):
    """Custom free-dim AP on a [128, F] tile: dims = [(step, count), ...]."""
    base = t[:]
    ap = [list(base.ap[0])] + [[s, c] for (s, c) in dims]
    return bass.AP(base.tensor, base.offset + off, ap)


def build_nc(debug=False):
    nc = bacc.Bacc()
    tblg = nc.declare_dram_parameter("tblg", [NTOT, REC], F32, isOutput=False)
    # per-center-type c_table rows, [d, t', k] order: 4 rows of 256 bf16
    BF16 = mybir.dt.bfloat16
    c2rA = nc.declare_dram_parameter("c2rA", [N_TYPES, 256], BF16,
                                     isOutput=False)
    ctr_d = nc.declare_dram_parameter("ctr", [P, S * 4], F32, isOutput=False)
    g1i_d = nc.declare_dram_parameter("g1i", [P, NIDX // 16], I16,
                                      isOutput=False)
    tia_d = nc.declare_dram_parameter("tia", [P, CA // 16], I16,
                                      isOutput=False)
    out_d = nc.declare_dram_parameter("out", [P, S * N_DESC * L_MAX], F32,
                                      isOutput=True)

    with tile.TileContext(nc) as tc:
        with tc.tile_pool(name="main", bufs=1) as pool:
            consts = pool.tile([P, 1], F32)
            nc.vector.memset(consts[:, 0:1], math.pi / 2)
            nc.const_aps.aps[(F32, math.pi / 2)] = consts[:, 0:1]

            ctr = pool.tile([P, S * 4], F32)
            g1i = pool.tile([P, NIDX // 16], I16)
            tia = pool.tile([P, CA // 16], I16)
            nc.sync.dma_start(out=ctr[:], in_=ctr_d[:])
            nc.sync.dma_start(out=g1i[:], in_=g1i_d[:])
            nc.sync.dma_start(out=tia[:], in_=tia_d[:])

            # ---- gathers, chunked (dma_gather caps out above 2K indices
            # per call; single_packet=False spreads descriptors over the
            # SDMA engines) --------------------------------------------------
            CHUNKS = [2048] * 12 + [1024]
            tblj = pool.tile([P, PAIRS * REC], F32)
            off = 0
            for ch in CHUNKS:
                nc.gpsimd.dma_gather(
                    _ap(tblj, (off // P) * REC,
                        [(REC, ch // P), (1, REC)]),
                    tblg[:], g1i[:, off // 16:(off + ch) // 16],
                    ch, ch, REC, single_packet=False)
                off += ch
            # per-atom c_table[t_i] rows (512B each): [p, s, (d, t', k)] bf16
            c_all = pool.tile([P, S * 256], BF16)
            nc.gpsimd.dma_gather(
                _ap(c_all, 0, [(256, S), (1, 256)]),
                c2rA[:], tia[:], CA, CA, 256, single_packet=False)

            # ---- pair geometry ---------------------------------------------
            # dxyz = tblj.xyz - ctr.xyz (broadcast over j)
            dxyz = pool.tile([P, PAIRS * 3], F32)
            nc.vector.tensor_tensor(
                out=_ap(dxyz, 0, [(60, S), (3, M_NBR), (1, 3)]),
                in0=_ap(tblj, 0, [(REC * M_NBR, S), (REC, M_NBR), (1, 3)]),
                in1=_ap(ctr, 0, [(4, S), (0, M_NBR), (1, 3)]),
                op=mybir.AluOpType.subtract)
            sq = pool.tile([P, PAIRS * 3], F32)
            nc.vector.tensor_tensor(out=sq[:], in0=dxyz[:], in1=dxyz[:],
                                    op=mybir.AluOpType.mult)
            r2 = pool.tile([P, PAIRS], F32)
            nc.vector.tensor_reduce(
                out=r2[:], in_=_ap(sq, 0, [(3, PAIRS), (1, 3)]),
                axis=mybir.AxisListType.X, op=mybir.AluOpType.add)
            r = pool.tile([P, PAIRS], F32)
            nc.scalar.sqrt(out=r[:], in_=r2[:])
            rinv = pool.tile([P, PAIRS], F32)
            nc.vector.reciprocal(out=rinv[:], in_=r[:])
            u = pool.tile([P, PAIRS * 3], F32)
            nc.vector.tensor_tensor(
                out=u[:], in0=dxyz[:],
                in1=_ap(rinv, 0, [(20, S), (1, M_NBR), (0, 3)]),
                op=mybir.AluOpType.mult)

            # ---- Chebyshev radial basis ------------------------------------
            s01 = pool.tile([P, PAIRS], F32)
            nc.vector.tensor_scalar_mul(out=s01[:], in0=r[:], scalar1=1.0 / R_C)
            # cos(pi*s) = sin(pi/2 - pi*s); arg stays in [-2.9, pi/2]
            cosx = pool.tile([P, PAIRS], F32)
            nc.scalar.activation(out=cosx[:], in_=s01[:],
                                 func=mybir.ActivationFunctionType.Sin,
                                 bias=math.pi / 2, scale=-math.pi)
            mask = pool.tile([P, PAIRS], F32)
            nc.vector.tensor_scalar(out=mask[:], in0=r[:], scalar1=R_C,
                                    scalar2=None, op0=mybir.AluOpType.is_lt)
            fch = pool.tile([P, PAIRS], F32)   # = fc/2, with cutoff mask
            tmp0 = pool.tile([P, PAIRS], F32)
            nc.vector.tensor_scalar(out=tmp0[:], in0=cosx[:], scalar1=0.25,
                                    scalar2=0.25, op0=mybir.AluOpType.mult,
                                    op1=mybir.AluOpType.add)
            nc.vector.tensor_tensor(out=fch[:], in0=tmp0[:], in1=mask[:],
                                    op=mybir.AluOpType.mult)
            tm1 = pool.tile([P, PAIRS], F32)
            nc.vector.tensor_scalar(out=tm1[:], in0=s01[:], scalar1=-1.0,
                                    scalar2=None, op0=mybir.AluOpType.add)
            xc = pool.tile([P, PAIRS], F32)
            nc.vector.scalar_tensor_tensor(out=xc[:], in0=tm1[:], scalar=2.0,
                                           in1=tm1[:], op0=mybir.AluOpType.mult,
                                           op1=mybir.AluOpType.mult)
            nc.vector.tensor_scalar(out=xc[:], in0=xc[:], scalar1=-1.0,
                                    scalar2=None, op0=mybir.AluOpType.add)
            x2 = pool.tile([P, PAIRS], F32)
            nc.vector.tensor_scalar_mul(out=x2[:], in0=xc[:], scalar1=2.0)

            f = pool.tile([P, PAIRS * K_MAX], F32)  # [s, j, k]

            def f_slice(k):
                return _ap(f, k, [(M_NBR * K_MAX, S), (K_MAX, M_NBR)])

            # f0 = 2*fch ; f1 = (xc+1)*fch ; fk = (Tk+1)*fch
            nc.vector.tensor_scalar_mul(out=f_slice(0), in0=fch[:], scalar1=2.0)
            nc.vector.scalar_tensor_tensor(out=f_slice(1), in0=xc[:], scalar=1.0,
                                           in1=fch[:], op0=mybir.AluOpType.add,
                                           op1=mybir.AluOpType.mult)
            # T2 = 2*xc^2 - 1
            Ta = pool.tile([P, PAIRS], F32)
            nc.vector.scalar_tensor_tensor(out=Ta[:], in0=xc[:], scalar=2.0,
                                           in1=xc[:], op0=mybir.AluOpType.mult,
                                           op1=mybir.AluOpType.mult)
            nc.vector.tensor_scalar(out=Ta[:], in0=Ta[:], scalar1=-1.0,
                                    scalar2=None, op0=mybir.AluOpType.add)
            nc.vector.scalar_tensor_tensor(out=f_slice(2), in0=Ta[:], scalar=1.0,
                                           in1=fch[:], op0=mybir.AluOpType.add,
                                           op1=mybir.AluOpType.mult)
            Tprev, Tprev2 = Ta, xc  # T2, T1
            extra = [pool.tile([P, PAIRS], F32, name=f"cheb{k}")
                     for k in range(5)]
            for k in range(3, K_MAX):
                Tk = extra[k - 3]
                nc.vector.tensor_tensor(out=Tk[:], in0=x2[:], in1=Tprev[:],
                                        op=mybir.AluOpType.mult)
                nc.vector.tensor_tensor(out=Tk[:], in0=Tk[:], in1=Tprev2[:],
                                        op=mybir.AluOpType.subtract)
                nc.vector.scalar_tensor_tensor(out=f_slice(k), in0=Tk[:],
                                               scalar=1.0, in1=fch[:],
                                               op0=mybir.AluOpType.add,
                                               op1=mybir.AluOpType.mult)
                Tprev, Tprev2 = Tk, Tprev

            # ---- g[d] = sum_{t',k} c_all[d,t',k] * oh4(t_j)[t'] * f[k] ------
            # iota4 constant row [0,1,2,3] per partition
            iota4 = pool.tile([P, 4], F32)
            for t in range(4):
                nc.vector.memset(iota4[:, t:t + 1], float(t))
            oh4 = pool.tile([P, PAIRS * 4], F32)  # [s, j, t']
            nc.vector.tensor_tensor(
                out=oh4[:],
                in0=_ap(tblj, 3, [(REC * M_NBR, S), (REC, M_NBR), (0, 4)]),
                in1=_ap(iota4, 0, [(0, S), (0, M_NBR), (1, 4)]),
                op=mybir.AluOpType.is_equal)
            F4 = pool.tile([P, PAIRS * 32], BF16)  # [s, j, t', k]
            nc.vector.tensor_tensor(
                out=F4[:],
                in0=_ap(oh4, 0, [(4 * M_NBR, S), (4, M_NBR), (1, 4),
                                 (0, K_MAX)]),
                in1=_ap(f, 0, [(K_MAX * M_NBR, S), (K_MAX, M_NBR),
                               (0, 4), (1, K_MAX)]),
                op=mybir.AluOpType.mult)
            g = pool.tile([P, PAIRS * N_DESC], F32)  # [s, j, d]
            with tc.tile_pool(name="x2p", bufs=2) as x2pool:
                for s in range(S):
                    x2t = x2pool.tile([P, M_NBR * N_DESC * 32], BF16,
                                      tag="x2t", name="x2t")
                    nc.vector.tensor_tensor(
                        out=x2t[:],
                        in0=_ap(c_all, s * 256,
                                [(0, M_NBR), (32, N_DESC), (1, 32)]),
                        in1=_ap(F4, s * M_NBR * 32,
                                [(32, M_NBR), (0, N_DESC), (1, 32)]),
                        op=mybir.AluOpType.mult)
                    nc.vector.tensor_reduce(
                        out=_ap(g, s * M_NBR * N_DESC,
                                [(N_DESC, M_NBR), (1, N_DESC)]),
                        in_=_ap(x2t, 0, [(N_DESC * 32, M_NBR), (32, N_DESC),
                                         (1, 32)]),
                        axis=mybir.AxisListType.X, op=mybir.AluOpType.add)

            # ---- spherical harmonics Y[16] ---------------------------------
            Y = pool.tile([P, PAIRS * 16], F32)  # [s, j, m]

            def y_slice(m, cnt=1):
                return _ap(Y, m, [(16 * M_NBR, S), (16, M_NBR), (1, cnt)])

            nc.vector.memset(y_slice(0), 1.0)
            nc.vector.tensor_copy(
                out=y_slice(1, 3),
                in_=_ap(u, 0, [(60, S), (3, M_NBR), (1, 3)]))

            def u_c(c):
                return _ap(u, c, [(60, S), (3, M_NBR)])

            x2c = pool.tile([P, PAIRS], F32)
            y2c = pool.tile([P, PAIRS], F32)
            z2c = pool.tile([P, PAIRS], F32)
            nc.scalar.square(out=x2c[:], in_=u_c(0))
            nc.scalar.square(out=y2c[:], in_=u_c(1))
            nc.scalar.square(out=z2c[:], in_=u_c(2))
            xyc = pool.tile([P, PAIRS], F32)
            nc.vector.tensor_tensor(out=xyc[:], in0=u_c(0), in1=u_c(1),
                                    op=mybir.AluOpType.mult)
            nc.vector.tensor_scalar_mul(out=y_slice(4), in0=xyc[:], scalar1=SQ3)
            nc.vector.scalar_tensor_tensor(out=y_slice(5), in0=u_c(1), scalar=SQ3,
                                           in1=u_c(2), op0=mybir.AluOpType.mult,
                                           op1=mybir.AluOpType.mult)
            nc.vector.scalar_tensor_tensor(out=y_slice(6), in0=u_c(0), scalar=SQ3,
                                           in1=u_c(2), op0=mybir.AluOpType.mult,
                                           op1=mybir.AluOpType.mult)
            nc.vector.tensor_scalar(out=y_slice(7), in0=z2c[:], scalar1=1.5,
                                    scalar2=-0.5, op0=mybir.AluOpType.mult,
                                    op1=mybir.AluOpType.add)
            dxyc = pool.tile([P, PAIRS], F32)
            nc.vector.tensor_tensor(out=dxyc[:], in0=x2c[:], in1=y2c[:],
                                    op=mybir.AluOpType.subtract)
            nc.vector.tensor_scalar_mul(out=y_slice(8), in0=dxyc[:],
                                        scalar1=SQ3 / 2)
            tl3 = pool.tile([P, PAIRS], F32)
            nc.vector.tensor_scalar(out=tl3[:], in0=z2c[:], scalar1=2.5,
                                    scalar2=-1.5, op0=mybir.AluOpType.mult,
                                    op1=mybir.AluOpType.add)
            nc.vector.tensor_tensor(out=y_slice(9), in0=tl3[:], in1=u_c(2),
                                    op=mybir.AluOpType.mult)
            tl4 = pool.tile([P, PAIRS], F32)
            nc.vector.tensor_scalar(out=tl4[:], in0=z2c[:], scalar1=5.0 * C31,
                                    scalar2=-C31, op0=mybir.AluOpType.mult,
                                    op1=mybir.AluOpType.add)
            nc.vector.tensor_tensor(out=y_slice(10), in0=tl4[:], in1=u_c(0),
                                    op=mybir.AluOpType.mult)
            nc.vector.tensor_tensor(out=y_slice(11), in0=tl4[:], in1=u_c(1),
                                    op=mybir.AluOpType.mult)
            nc.vector.scalar_tensor_tensor(out=y_slice(12), in0=dxyc[:],
                                           scalar=C32 / 2, in1=u_c(2),
                                           op0=mybir.AluOpType.mult,
                                           op1=mybir.AluOpType.mult)
            nc.vector.scalar_tensor_tensor(out=y_slice(13), in0=xyc[:],
                                           scalar=C32, in1=u_c(2),
                                           op0=mybir.AluOpType.mult,
                                           op1=mybir.AluOpType.mult)
            tl5 = pool.tile([P, PAIRS], F32)
            nc.vector.scalar_tensor_tensor(out=tl5[:], in0=y2c[:], scalar=3.0,
                                           in1=x2c[:], op0=mybir.AluOpType.mult,
                                           op1=mybir.AluOpType.subtract)
            nc.vector.scalar_tensor_tensor(out=y_slice(14), in0=tl5[:],
                                           scalar=-C33, in1=u_c(0),
                                           op0=mybir.AluOpType.mult,
                                           op1=mybir.AluOpType.mult)
            tl6 = pool.tile([P, PAIRS], F32)
            nc.vector.scalar_tensor_tensor(out=tl6[:], in0=x2c[:], scalar=3.0,
                                           in1=y2c[:], op0=mybir.AluOpType.mult,
                                           op1=mybir.AluOpType.subtract)
            nc.vector.scalar_tensor_tensor(out=y_slice(15), in0=tl6[:],
                                           scalar=C33, in1=u_c(1),
                                           op0=mybir.AluOpType.mult,
                                           op1=mybir.AluOpType.mult)

            # ---- A[s,d,m] = sum_j g*Y ; B[s,d] = sum_j g^2 ------------------
            A = pool.tile([P, S * N_DESC * 16], F32)  # [s, d, m]
            with tc.tile_pool(name="xap", bufs=2) as xapool:
                for s in range(S):
                    xa = xapool.tile([P, M_NBR * N_DESC * 16], F32, tag="xa",
                                     name="xa")
                    nc.vector.tensor_tensor(
                        out=xa[:],
                        in0=_ap(g, s * M_NBR * N_DESC,
                                [(N_DESC, M_NBR), (1, N_DESC), (0, 16)]),
                        in1=_ap(Y, s * M_NBR * 16,
                                [(16, M_NBR), (0, N_DESC), (1, 16)]),
                        op=mybir.AluOpType.mult)
                    nc.vector.tensor_reduce(
                        out=_ap(A, s * N_DESC * 16, [(1, N_DESC * 16)]),
                        in_=_ap(xa, 0, [(16, N_DESC), (1, 16),
                                        (N_DESC * 16, M_NBR)]),
                        axis=mybir.AxisListType.X, op=mybir.AluOpType.add)

            gsq = pool.tile([P, PAIRS * N_DESC], F32)
            nc.vector.tensor_tensor(out=gsq[:], in0=g[:], in1=g[:],
                                    op=mybir.AluOpType.mult)
            Bh = pool.tile([P, S * N_DESC], F32)
            nc.vector.tensor_reduce(
                out=Bh[:],
                in_=_ap(gsq, 0, [(M_NBR * N_DESC, S), (1, N_DESC),
                                 (N_DESC, M_NBR)]),
                axis=mybir.AxisListType.X, op=mybir.AluOpType.add)
            nc.vector.tensor_scalar_mul(out=Bh[:], in0=Bh[:], scalar1=0.5)

            # ---- q[s,d,l] = 0.5*sum_{m in l} A^2 - Bh ----------------------
            Asq = pool.tile([P, S * N_DESC * 16], F32)
            nc.vector.tensor_tensor(out=Asq[:], in0=A[:], in1=A[:],
                                    op=mybir.AluOpType.mult)
            outq = pool.tile([P, S * N_DESC * L_MAX], F32)  # [s, d, l]
            q2l = pool.tile([P, S * N_DESC], F32)
            for l in range(L_MAX):
                cnt = SHELL_OFF[l + 1] - SHELL_OFF[l]
                nc.vector.tensor_reduce(
                    out=q2l[:],
                    in_=_ap(Asq, SHELL_OFF[l],
                            [(N_DESC * 16, S), (16, N_DESC), (1, cnt)]),
                    axis=mybir.AxisListType.X, op=mybir.AluOpType.add)
                nc.vector.scalar_tensor_tensor(
                    out=_ap(outq, l, [(N_DESC * L_MAX, S), (L_MAX, N_DESC)]),
                    in0=q2l[:], scalar=0.5, in1=Bh[:],
                    op0=mybir.AluOpType.mult, op1=mybir.AluOpType.subtract)

            nc.sync.dma_start(out=out_d[:], in_=outq[:])

            if debug:
                for nm, t in [("d_tblj", tblj), ("d_call", c_all), ("d_r", r),
                              ("d_fch", fch), ("d_f", f), ("d_g", g),
                              ("d_Y", Y), ("d_A", A), ("d_u", u)]:
                    dd = nc.declare_dram_parameter(
                        nm, [P, t.shape[1]], F32, isOutput=True)
                    nc.sync.dma_start(out=dd[:], in_=t[:])
    nc.finalize()
    return nc


def make_inputs(types, positions, angular_neighbors, c_table):
    """Host-side marshaling: packed gather table, per-core index tiles."""
    types = np.asarray(types).astype(np.int64)
    positions = np.ascontiguousarray(np.asarray(positions, dtype=np.float32))
    nbr = np.asarray(angular_neighbors).astype(np.int64)
    c_table = np.asarray(c_table, dtype=np.float32)

    pad = NTOT - N_ATOMS
    types_pad = np.concatenate([types, np.repeat(types[-1:], pad, 0)], 0)
    pos_pad = np.concatenate([positions, np.repeat(positions[-1:], pad, 0)], 0)
    nbr_pad = np.concatenate([nbr, np.repeat(nbr[-1:], pad, 0)], 0)

    tblg = np.zeros((NTOT, REC), dtype=np.float32)
    tblg[:, :3] = pos_pad
    tblg[:, 3] = types_pad
    # per-center-type rows in [d, t', k] order
    c2rA = np.ascontiguousarray(
        c_table.transpose(0, 2, 1, 3).reshape(N_TYPES, 256))

    import ml_dtypes
    c2rA = c2rA.astype(ml_dtypes.bfloat16)

    # pair linear order: i = (s*20+j)*128 + p ; atom = base + p*10 + s
    pvec = np.arange(P)
    svec = np.arange(S)
    jvec = np.arange(M_NBR)
    # [S, M, P] -> flat i order (c-major, then p)
    atom_l = (pvec[None, None, :] * S + svec[:, None, None])  # local atom id

    def wrap16(lin):
        w = lin.reshape(-1, 16).T.astype(np.int16)               # [16, n/16]
        return np.ascontiguousarray(np.tile(w, (8, 1)))          # [128, ...]

    in_maps = []
    for c in range(NCORES):
        al = atom_l + c * CA                                     # global atoms
        nb = nbr_pad[al, jvec[None, :, None]]                    # [S, M, P]
        g1_lin = nb.reshape(-1)
        # c_all gather: record i -> (p=i%128, s=i//128); index = types[atom]
        tia_lin = types_pad[(np.arange(CA) % P) * S +
                            (np.arange(CA) // P) + c * CA]
        ctr_core = tblg[c * CA:(c + 1) * CA, :4].reshape(P, S, 4)
        # ctr layout wants [p, s, 4]; rows of tblg are atom = p*10+s already
        in_maps.append({
            "tblg": tblg,
            "c2rA": c2rA,
            "ctr": np.ascontiguousarray(ctr_core.reshape(P, S * 4)),
            "g1i": wrap16(g1_lin),
            "tia": wrap16(tia_lin),
        })
    return in_maps


_NC_CACHE = None


def kernel(types, positions, angular_neighbors, c_table):
    global _NC_CACHE
    in_maps = make_inputs(types, positions, angular_neighbors, c_table)
    if _NC_CACHE is None:
        _NC_CACHE = build_nc()
    res = run_bass_kernel_spmd(_NC_CACHE, in_maps,
                               core_ids=list(range(NCORES)))
    outs = [res.results[c]["out"].reshape(CA, N_DESC, L_MAX)
            for c in range(NCORES)]
    q = np.concatenate(outs, 0)[:N_ATOMS]
    return np.ascontiguousarray(q)


if __name__ == "__main__":
    import os
    if os.path.exists("/tmp/ref_cache.npz"):
        z = np.load("/tmp/ref_cache.npz")
        inputs = {k: z[k] for k in
                  ("types", "positions", "angular_neighbors", "c_table")}
        exp = z["exp"]
    else:
        import reference
        inputs = {k: np.asarray(v) for k, v in reference.setup_inputs().items()}
        exp = np.asarray(reference.reference(**inputs))
    act = kernel(**inputs)
    rel = np.linalg.norm(act - exp) / np.linalg.norm(exp)
    print("Relative error:", rel)



# revision 3
# speedup vs baseline: 4.6840x; 4.6840x over previous
"""Trainium2 Bass kernel for nn_AngularDescriptor (gnn_message_passing).

Legendre addition theorem factorization (as baseline):
  q[i,d,l] = 0.5 * ( sum_{m in shell l} A[i,d,m]^2  -  B[i,d] )
  A[i,d,m] = sum_j g_ij[d] * Yhat_m(u_ij),   B[i,d] = sum_j g_ij[d]^2

v2: all index-based gathers moved to host marshaling (dense per-core
slabs: per-pair neighbor positions and per-pair c_table[t_i,t_j] rows in
bf16).  Device does the FP math only.  g contraction over k uses a bf16
2x-mode multiply + reduction tree.
"""
import os
import sys

sys.path.insert(0, "/opt/trn_rl_repo")
os.environ.setdefault("NEURON_RT_RESET_CORES", "1")

import math
import numpy as np

from concourse import bacc, bass, mybir, tile
from concourse.bass_utils import run_bass_kernel_spmd

# problem constants (hardcoded per harness rules)
N_ATOMS = 10000
M_NBR = 20
N_TYPES = 4
N_DESC = 8
K_MAX = 8
L_MAX = 4
R_C = 5.0

NCORES = 8
P = 128
S = 10                       # atom slots per partition
CA = P * S                   # atoms per core = 1280
NTOT = NCORES * CA           # padded atom count = 10240
PAIRS = S * M_NBR            # 200 pair slots per partition
F32 = mybir.dt.float32
BF16 = mybir.dt.bfloat16

SQ3 = math.sqrt(3.0)
C31 = math.sqrt(3.0 / 8.0)
C32 = math.sqrt(15.0)
C33 = math.sqrt(5.0 / 8.0)
SHELL_OFF = [0, 1, 4, 9, 16]


def _ap(t, off, dims):
    """Custom free-dim AP on a [128, F] tile: dims = [(step, count), ...]."""
    base = t[:]
    ap = [list(base.ap[0])] + [[s, c] for (s, c) in dims]
    return bass.AP(base.tensor, base.offset + off, ap)


def build_nc(debug=False):
    nc = bacc.Bacc()
    ctr_d = nc.declare_dram_parameter("ctr", [P, S * 3], F32, isOutput=False)
    posj_d = nc.declare_dram_parameter("posj", [P, PAIRS * 3], F32,
                                       isOutput=False)
    cpair_d = nc.declare_dram_parameter("cpair", [P, PAIRS * 64], BF16,
                                        isOutput=False)
    out_d = nc.declare_dram_parameter("out", [P, S * N_DESC * L_MAX], F32,
                                      isOutput=True)

    with tile.TileContext(nc) as tc:
        with tc.tile_pool(name="main", bufs=1) as pool:
            consts = pool.tile([P, 1], F32)
            nc.vector.memset(consts[:, 0:1], math.pi / 2)
            nc.const_aps.aps[(F32, math.pi / 2)] = consts[:, 0:1]

            ctr = pool.tile([P, S * 3], F32)
            posj = pool.tile([P, PAIRS * 3], F32)
            cpair = pool.tile([P, PAIRS * 64], BF16)
            nc.sync.dma_start(out=ctr[:], in_=ctr_d[:])
            nc.sync.dma_start(out=posj[:], in_=posj_d[:])
            nc.sync.dma_start(out=cpair[:], in_=cpair_d[:])

            # ---- pair geometry ---------------------------------------------
            dxyz = pool.tile([P, PAIRS * 3], F32)
            nc.vector.tensor_tensor(
                out=_ap(dxyz, 0, [(60, S), (3, M_NBR), (1, 3)]),
                in0=_ap(posj, 0, [(60, S), (3, M_NBR), (1, 3)]),
                in1=_ap(ctr, 0, [(3, S), (0, M_NBR), (1, 3)]),
                op=mybir.AluOpType.subtract)
            sq = pool.tile([P, PAIRS * 3], F32)
            nc.scalar.square(out=sq[:], in_=dxyz[:])
            r2 = pool.tile([P, PAIRS], F32)
            nc.vector.tensor_reduce(
                out=r2[:], in_=_ap(sq, 0, [(3, PAIRS), (1, 3)]),
                axis=mybir.AxisListType.X, op=mybir.AluOpType.add)
            r = pool.tile([P, PAIRS], F32)
            nc.scalar.sqrt(out=r[:], in_=r2[:])
            rinv = pool.tile([P, PAIRS], F32)
            nc.vector.reciprocal(out=rinv[:], in_=r[:])
            u = pool.tile([P, PAIRS * 3], F32)
            nc.vector.tensor_tensor(
                out=u[:], in0=dxyz[:],
                in1=_ap(rinv, 0, [(20, S), (1, M_NBR), (0, 3)]),
                op=mybir.AluOpType.mult)

            # ---- Chebyshev radial basis ------------------------------------
            s01 = pool.tile([P, PAIRS], F32)
            nc.vector.tensor_scalar_mul(out=s01[:], in0=r[:], scalar1=1.0 / R_C)
            # cos(pi*s) = sin(pi/2 - pi*s); arg stays in [-2.9, pi/2]
            cosx = pool.tile([P, PAIRS], F32)
            nc.scalar.activation(out=cosx[:], in_=s01[:],
                                 func=mybir.ActivationFunctionType.Sin,
                                 bias=math.pi / 2, scale=-math.pi)
            mask = pool.tile([P, PAIRS], F32)
            nc.vector.tensor_scalar(out=mask[:], in0=r[:], scalar1=R_C,
                                    scalar2=None, op0=mybir.AluOpType.is_lt)
            fch = pool.tile([P, PAIRS], F32)   # = fc/2, with cutoff mask
            tmp0 = pool.tile([P, PAIRS], F32)
            nc.vector.tensor_scalar(out=tmp0[:], in0=cosx[:], scalar1=0.25,
                                    scalar2=0.25, op0=mybir.AluOpType.mult,
                                    op1=mybir.AluOpType.add)
            nc.vector.tensor_tensor(out=fch[:], in0=tmp0[:], in1=mask[:],
                                    op=mybir.AluOpType.mult)
            tm1 = pool.tile([P, PAIRS], F32)
            nc.vector.tensor_scalar(out=tm1[:], in0=s01[:], scalar1=-1.0,
                                    scalar2=None, op0=mybir.AluOpType.add)
            xc = pool.tile([P, PAIRS], F32)
            nc.vector.scalar_tensor_tensor(out=xc[:], in0=tm1[:], scalar=2.0,
                                           in1=tm1[:], op0=mybir.AluOpType.mult,
                                           op1=mybir.AluOpType.mult)
            nc.vector.tensor_scalar(out=xc[:], in0=xc[:], scalar1=-1.0,
                                    scalar2=None, op0=mybir.AluOpType.add)
            x2 = pool.tile([P, PAIRS], F32)
            nc.vector.tensor_scalar_mul(out=x2[:], in0=xc[:], scalar1=2.0)

            # f[sj, k] (k innermost), bf16 for the 2x g-multiply
            f = pool.tile([P, PAIRS * K_MAX], BF16)

            def f_slice(k):
                return _ap(f, k, [(K_MAX, PAIRS)])

            # f0 = 2*fch ; f1 = (xc+1)*fch ; fk = (Tk+1)*fch
            nc.vector.tensor_scalar_mul(out=f_slice(0), in0=fch[:], scalar1=2.0)
            nc.vector.scalar_tensor_tensor(out=f_slice(1), in0=xc[:], scalar=1.0,
                                           in1=fch[:], op0=mybir.AluOpType.add,
                                           op1=mybir.AluOpType.mult)
            # T2 = 2*xc^2 - 1
            Ta = pool.tile([P, PAIRS], F32)
            nc.vector.scalar_tensor_tensor(out=Ta[:], in0=xc[:], scalar=2.0,
                                           in1=xc[:], op0=mybir.AluOpType.mult,
                                           op1=mybir.AluOpType.mult)
            nc.vector.tensor_scalar(out=Ta[:], in0=Ta[:], scalar1=-1.0,
                                    scalar2=None, op0=mybir.AluOpType.add)
            nc.vector.scalar_tensor_tensor(out=f_slice(2), in0=Ta[:], scalar=1.0,
                                           in1=fch[:], op0=mybir.AluOpType.add,
                                           op1=mybir.AluOpType.mult)
            Tprev, Tprev2 = Ta, xc  # T2, T1
            extra = [pool.tile([P, PAIRS], F32, name=f"cheb{k}")
                     for k in range(5)]
            for k in range(3, K_MAX):
                Tk = extra[k - 3]
                nc.vector.tensor_tensor(out=Tk[:], in0=x2[:], in1=Tprev[:],
                                        op=mybir.AluOpType.mult)
                nc.vector.tensor_tensor(out=Tk[:], in0=Tk[:], in1=Tprev2[:],
                                        op=mybir.AluOpType.subtract)
                nc.vector.scalar_tensor_tensor(out=f_slice(k), in0=Tk[:],
                                               scalar=1.0, in1=fch[:],
                                               op0=mybir.AluOpType.add,
                                               op1=mybir.AluOpType.mult)
                Tprev, Tprev2 = Tk, Tprev

            # ---- g[sj, d] = sum_k cpair[sj, d, k] * f[sj, k] ----------------
            # multiply in bf16 2x mode, then a k-reduction tree (8->4->2->1)
            gtmp = pool.tile([P, PAIRS * 64], BF16)   # [sj, d, k]
            nc.vector.tensor_tensor(
                out=_ap(gtmp, 0, [(64, PAIRS), (8, N_DESC), (1, K_MAX)]),
                in0=_ap(cpair, 0, [(64, PAIRS), (8, N_DESC), (1, K_MAX)]),
                in1=_ap(f, 0, [(K_MAX, PAIRS), (0, N_DESC), (1, K_MAX)]),
                op=mybir.AluOpType.mult)
            t1 = pool.tile([P, PAIRS * 32], BF16)     # [sj, d, 4]
            nc.vector.tensor_tensor(
                out=_ap(t1, 0, [(4, PAIRS * N_DESC), (1, 4)]),
                in0=_ap(gtmp, 0, [(8, PAIRS * N_DESC), (1, 4)]),
                in1=_ap(gtmp, 4, [(8, PAIRS * N_DESC), (1, 4)]),
                op=mybir.AluOpType.add)
            t2 = pool.tile([P, PAIRS * 16], BF16)     # [sj, d, 2]
            nc.vector.tensor_tensor(
                out=_ap(t2, 0, [(2, PAIRS * N_DESC), (1, 2)]),
                in0=_ap(t1, 0, [(4, PAIRS * N_DESC), (1, 2)]),
                in1=_ap(t1, 2, [(4, PAIRS * N_DESC), (1, 2)]),
                op=mybir.AluOpType.add)
            g = pool.tile([P, PAIRS * N_DESC], F32)   # [sj, d]
            nc.vector.tensor_tensor(
                out=g[:],
                in0=_ap(t2, 0, [(2, PAIRS * N_DESC)]),
                in1=_ap(t2, 1, [(2, PAIRS * N_DESC)]),
                op=mybir.AluOpType.add)

            # ---- spherical harmonics Y[16] ---------------------------------
            Y = pool.tile([P, PAIRS * 16], F32)  # [sj, m]

            def y_slice(m, cnt=1):
                return _ap(Y, m, [(16 * M_NBR, S), (16, M_NBR), (1, cnt)])

            nc.vector.memset(y_slice(0), 1.0)
            nc.vector.tensor_copy(
                out=y_slice(1, 3),
                in_=_ap(u, 0, [(60, S), (3, M_NBR), (1, 3)]))

            def u_c(c):
                return _ap(u, c, [(60, S), (3, M_NBR)])

            x2c = pool.tile([P, PAIRS], F32)
            y2c = pool.tile([P, PAIRS], F32)
            z2c = pool.tile([P, PAIRS], F32)
            nc.scalar.square(out=x2c[:], in_=u_c(0))
            nc.scalar.square(out=y2c[:], in_=u_c(1))
            nc.scalar.square(out=z2c[:], in_=u_c(2))
            xyc = pool.tile([P, PAIRS], F32)
            nc.vector.tensor_tensor(out=xyc[:], in0=u_c(0), in1=u_c(1),
                                    op=mybir.AluOpType.mult)
            nc.vector.tensor_scalar_mul(out=y_slice(4), in0=xyc[:], scalar1=SQ3)
            nc.vector.scalar_tensor_tensor(out=y_slice(5), in0=u_c(1), scalar=SQ3,
                                           in1=u_c(2), op0=mybir.AluOpType.mult,
                                           op1=mybir.AluOpType.mult)
            nc.vector.scalar_tensor_tensor(out=y_slice(6), in0=u_c(0), scalar=SQ3,
                                           in1=u_c(2), op0=mybir.AluOpType.mult,
                                           op1=mybir.AluOpType.mult)
            nc.vector.tensor_scalar(out=y_slice(7), in0=z2c[:], scalar1=1.5,
                                    scalar2=-0.5, op0=mybir.AluOpType.mult,
                                    op1=mybir.AluOpType.add)
            dxyc = pool.tile([P, PAIRS], F32)
            nc.vector.tensor_tensor(out=dxyc[:], in0=x2c[:], in1=y2c[:],
                                    op=mybir.AluOpType.subtract)
            nc.vector.tensor_scalar_mul(out=y_slice(8), in0=dxyc[:],
                                        scalar1=SQ3 / 2)
            tl3 = pool.tile([P, PAIRS], F32)
            nc.vector.tensor_scalar(out=tl3[:], in0=z2c[:], scalar1=2.5,
                                    scalar2=-1.5, op0=mybir.AluOpType.mult,
                                    op1=mybir.AluOpType.add)
            nc.vector.tensor_tensor(out=y_slice(9), in0=tl3[:], in1=u_c(2),
                                    op=mybir.AluOpType.mult)
            tl4 = pool.tile([P, PAIRS], F32)
            nc.vector.tensor_scalar(out=tl4[:], in0=z2c[:], scalar1=5.0 * C31,
                                    scalar2=-C31, op0=mybir.AluOpType.mult,
                                    op1=mybir.AluOpType.add)
            nc.vector.tensor_tensor(out=y_slice(10), in0=tl4[:], in1=u_c(0),
                                    op=mybir.AluOpType.mult)
            nc.vector.tensor_tensor(out=y_slice(11), in0=tl4[:], in1=u_c(1),
                                    op=mybir.AluOpType.mult)
            nc.vector.scalar_tensor_tensor(out=y_slice(12), in0=dxyc[:],
                                           scalar=C32 / 2, in1=u_c(2),
                                           op0=mybir.AluOpType.mult,
                                           op1=mybir.AluOpType.mult)
            nc.vector.scalar_tensor_tensor(out=y_slice(13), in0=xyc[:],
                                           scalar=C32, in1=u_c(2),
                                           op0=mybir.AluOpType.mult,
                                           op1=mybir.AluOpType.mult)
            tl5 = pool.tile([P, PAIRS], F32)
            nc.vector.scalar_tensor_tensor(out=tl5[:], in0=y2c[:], scalar=3.0,
                                           in1=x2c[:], op0=mybir.AluOpType.mult,
                                           op1=mybir.AluOpType.subtract)
            nc.vector.scalar_tensor_tensor(out=y_slice(14), in0=tl5[:],
                                           scalar=-C33, in1=u_c(0),
                                           op0=mybir.AluOpType.mult,
                                           op1=mybir.AluOpType.mult)
            tl6 = pool.tile([P, PAIRS], F32)
            nc.vector.scalar_tensor_tensor(out=tl6[:], in0=x2c[:], scalar=3.0,
                                           in1=y2c[:], op0=mybir.AluOpType.mult,
                                           op1=mybir.AluOpType.subtract)
            nc.vector.scalar_tensor_tensor(out=y_slice(15), in0=tl6[:],
                                           scalar=C33, in1=u_c(1),
                                           op0=mybir.AluOpType.mult,
                                           op1=mybir.AluOpType.mult)

            # ---- A[s,d,m] = sum_j g*Y ; B[s,d] = sum_j g^2 ------------------
            A = pool.tile([P, S * N_DESC * 16], F32)  # [s, d, m]
            NGP = 10  # s-slots whose xa multiply runs on gpsimd
            with tc.tile_pool(name="xap", bufs=2) as xapool:
                for s in range(S):
                    eng = nc.gpsimd if s >= S - NGP else nc.vector
                    xa = xapool.tile([P, M_NBR * N_DESC * 16], F32, tag="xa",
                                     name="xa")
                    eng.tensor_tensor(
                        out=xa[:],
                        in0=_ap(g, s * M_NBR * N_DESC,
                                [(N_DESC, M_NBR), (1, N_DESC), (0, 16)]),
                        in1=_ap(Y, s * M_NBR * 16,
                                [(16, M_NBR), (0, N_DESC), (1, 16)]),
                        op=mybir.AluOpType.mult)
                    nc.vector.tensor_reduce(
                        out=_ap(A, s * N_DESC * 16, [(1, N_DESC * 16)]),
                        in_=_ap(xa, 0, [(16, N_DESC), (1, 16),
                                        (N_DESC * 16, M_NBR)]),
                        axis=mybir.AxisListType.X, op=mybir.AluOpType.add)

            gsq = pool.tile([P, PAIRS * N_DESC], F32)
            nc.scalar.square(out=gsq[:], in_=g[:])
            Bh = pool.tile([P, S * N_DESC], F32)
            nc.vector.tensor_reduce(
                out=Bh[:],
                in_=_ap(gsq, 0, [(M_NBR * N_DESC, S), (1, N_DESC),
                                 (N_DESC, M_NBR)]),
                axis=mybir.AxisListType.X, op=mybir.AluOpType.add)
            nc.vector.tensor_scalar_mul(out=Bh[:], in0=Bh[:], scalar1=0.5)

            # ---- q[s,d,l] = 0.5*sum_{m in l} A^2 - Bh ----------------------
            Asq = pool.tile([P, S * N_DESC * 16], F32)
            nc.scalar.square(out=Asq[:], in_=A[:])
            outq = pool.tile([P, S * N_DESC * L_MAX], F32)  # [s, d, l]
            q2l = pool.tile([P, S * N_DESC], F32)
            for l in range(L_MAX):
                cnt = SHELL_OFF[l + 1] - SHELL_OFF[l]
                nc.vector.tensor_reduce(
                    out=q2l[:],
                    in_=_ap(Asq, SHELL_OFF[l],
                            [(N_DESC * 16, S), (16, N_DESC), (1, cnt)]),
                    axis=mybir.AxisListType.X, op=mybir.AluOpType.add)
                nc.vector.scalar_tensor_tensor(
                    out=_ap(outq, l, [(N_DESC * L_MAX, S), (L_MAX, N_DESC)]),
                    in0=q2l[:], scalar=0.5, in1=Bh[:],
                    op0=mybir.AluOpType.mult, op1=mybir.AluOpType.subtract)

            nc.sync.dma_start(out=out_d[:], in_=outq[:])

            if debug:
                for nm, t in [("d_r", r), ("d_f", f), ("d_g", g),
                              ("d_Y", Y), ("d_A", A), ("d_u", u),
                              ("d_gtmp", gtmp)]:
                    dd = nc.declare_dram_parameter(
                        nm, [P, t.shape[1]], F32, isOutput=True)
                    nc.sync.dma_start(out=dd[:], in_=t[:])
    nc.finalize()
    return nc


def make_inputs(types, positions, angular_neighbors, c_table):
    """Host-side marshaling: dense per-core slabs, no device gathers."""
    import ml_dtypes
    types = np.asarray(types).astype(np.int64)
    positions = np.ascontiguousarray(np.asarray(positions, dtype=np.float32))
    nbr = np.asarray(angular_neighbors).astype(np.int64)
    c_table = np.asarray(c_table, dtype=np.float32)

    pad = NTOT - N_ATOMS
    types_pad = np.concatenate([types, np.repeat(types[-1:], pad, 0)], 0)
    pos_pad = np.concatenate([positions, np.repeat(positions[-1:], pad, 0)], 0)
    nbr_pad = np.concatenate([nbr, np.repeat(nbr[-1:], pad, 0)], 0)

    # atom at (core c, partition p, slot s) = c*CA + p*S + s
    atom = (np.arange(P)[:, None] * S + np.arange(S)[None, :])  # [P, S]
    in_maps = []
    for c in range(NCORES):
        ga = atom + c * CA                                      # [P, S]
        nb = nbr_pad[ga]                                        # [P, S, M]
        posj = pos_pad[nb].reshape(P, PAIRS * 3)                # [P,S,M,3]
        ctr = pos_pad[ga].reshape(P, S * 3)
        ti = types_pad[ga][:, :, None]                          # [P, S, 1]
        tj = types_pad[nb]                                      # [P, S, M]
        cpair = c_table[ti, tj]                                 # [P,S,M,8,8]
        in_maps.append({
            "ctr": np.ascontiguousarray(ctr),
            "posj": np.ascontiguousarray(posj),
            "cpair": np.ascontiguousarray(
                cpair.reshape(P, PAIRS * 64)).astype(ml_dtypes.bfloat16),
        })
    return in_maps


_NC_CACHE = None


def kernel(types, positions, angular_neighbors, c_table):
    global _NC_CACHE
    in_maps = make_inputs(types, positions, angular_neighbors, c_table)
    if _NC_CACHE is None:
        _NC_CACHE = build_nc()
    res = run_bass_kernel_spmd(_NC_CACHE, in_maps,
                               core_ids=list(range(NCORES)))
    outs = [res.results[c]["out"].reshape(CA, N_DESC, L_MAX)
            for c in range(NCORES)]
    q = np.concatenate(outs, 0)[:N_ATOMS]
    return np.ascontiguousarray(q)


if __name__ == "__main__":
    if os.path.exists("/tmp/ref_cache.npz"):
        z = np.load("/tmp/ref_cache.npz")
        inputs = {k: z[k] for k in
                  ("types", "positions", "angular_neighbors", "c_table")}
        exp = z["exp"]
    else:
        import reference
        inputs = {k: np.asarray(v) for k, v in reference.setup_inputs().items()}
        exp = np.asarray(reference.reference(**inputs))
    act = kernel(**inputs)
    rel = np.linalg.norm(act - exp) / np.linalg.norm(exp)
    print("Relative error:", rel)
